# revision 6
# baseline (speedup 1.0000x reference)
"""Trainium2 Bass kernel for nn_AttentionBlock (B=8, C=512, H=W=32, NH=8, DH=64).

Sharding: pure data-parallel - one batch element per NeuronCore (8 cores).

Per-core pipeline (channels-on-partitions, HW=1024 spatial), fp8-heavy:
  groupnorm (f32 stats, h cast to fp8e4)
  -> qkv 1x1conv as fp8 DoubleRow matmuls (K=256/pass, 2 cols/cycle)
  -> attention:
       scores S^T[j,i] = k_j . q_i as plain fp8 matmuls with the 64-dim
       head contraction zero-padded to K=128 (kA rows 64-127 = 0, kB
       rows 0-63 = 0): every weight load is a full 128x128 tile at a
       fixed PE position, so it pipelines behind the previous stream;
       exp on ScalarE (scale=1/8, bias=-2; the bias cancels in softmax
       and keeps exp outputs inside fp8e4 range) writing fp8 directly;
       attn.V as fp8 DoubleRow over spatial-tile pairs, V zero-padded
       to 128 columns with a ones-column at 64 giving row sums l;
       transpose via DMA xbar, normalize on VectorE -> h2 (fp8) via
       DRAM round-trip
  -> proj 1x1conv fp8 DoubleRow -> residual add (x + B2 bias fold).

ScalarE exp (64 x [128,1024] tiles, ~71us) is the pipeline floor. All
other tensor work (qkv conv leftovers, attn.V passes, normalize steps)
is threaded one matmul at a time into the slack between score matmuls
via a FIFO work queue (2 pops per exp), so the exp stream stays dense.
PSUM: psA = 2-deep ring of short-lived tiles (warmup/gn/conv/scores/
proj), psB = the two long-lived attn.V accumulators of the active pair.

Host-side: weights transposed + cast to fp8e4; v-bias and proj-bias
folded into B2[o, s%64] added into x (gpsimd) for the residual.
"""

import numpy as np
import ml_dtypes

import concourse.bass as bass
import concourse.mybir as mybir
import concourse.tile as tile
from concourse import bacc
from concourse.bass_utils import run_bass_kernel_spmd

F32 = mybir.dt.float32
BF16 = mybir.dt.bfloat16
F8 = mybir.dt.float8e4
DR = mybir.MatmulPerfMode.DoubleRow
NPF8 = ml_dtypes.float8_e4m3

B, C, HW = 8, 512, 1024
NH, DH = 8, 64
GROUPS, EPS = 32, 1e-5
CT = C // 128          # 4 channel tiles
ST = HW // 128         # 8 spatial tiles
GPT = 8                # groups per 128-channel tile
CPG = 16               # channels per group
EXPB = 2.0             # exp bias (cancels in softmax; keeps fp8 in range)

_CACHE: dict = {}


def _build():
    nc = bacc.Bacc("TRN2", target_bir_lowering=False, debug=False, num_devices=8)

    x_d = nc.declare_dram_parameter("x", [C, HW], F32, isOutput=False)
    wq_d = nc.declare_dram_parameter("wqkvT", [C, 3 * C], F8, isOutput=False)
    wp_d = nc.declare_dram_parameter("wprojT", [C, C], F8, isOutput=False)
    qkb_d = nc.declare_dram_parameter("qkb", [2 * C], F32, isOutput=False)
    gam_d = nc.declare_dram_parameter("gamma", [C], F32, isOutput=False)
    bet_d = nc.declare_dram_parameter("beta", [C], F32, isOutput=False)
    G_d = nc.declare_dram_parameter("G", [128, GPT], F32, isOutput=False)
    GT_d = nc.declare_dram_parameter("GT", [8, 128], F32, isOutput=False)
    B2_d = nc.declare_dram_parameter("B2", [C, DH], F32, isOutput=False)
    out_d = nc.declare_dram_parameter("out", [C, HW], F32, isOutput=True)
    h2_d = nc.dram_tensor("h2d", [C, HW], F8)

    import bass_rust
    from contextlib import ExitStack

    with tile.TileContext(nc) as tc, ExitStack() as ctx:
        const = ctx.enter_context(tc.tile_pool(name="const", bufs=1))
        small = ctx.enter_context(tc.tile_pool(name="small", bufs=2))
        xp = ctx.enter_context(tc.tile_pool(name="xp", bufs=1))
        hp = ctx.enter_context(tc.tile_pool(name="hp", bufs=1))
        wqp = ctx.enter_context(tc.tile_pool(name="wqp", bufs=1))
        wpp = ctx.enter_context(tc.tile_pool(name="wpp", bufs=1))
        qp = ctx.enter_context(tc.tile_pool(name="qp", bufs=1))
        kpA = ctx.enter_context(tc.tile_pool(name="kpA", bufs=1))
        kpB = ctx.enter_context(tc.tile_pool(name="kpB", bufs=1))
        vpl = ctx.enter_context(tc.tile_pool(name="vpl", bufs=1))
        ptp = ctx.enter_context(tc.tile_pool(name="ptp", bufs=2))
        h2p = ctx.enter_context(tc.tile_pool(name="h2p", bufs=1))
        o2tp = ctx.enter_context(tc.tile_pool(name="o2tp", bufs=2))
        o2trp = ctx.enter_context(tc.tile_pool(name="o2trp", bufs=2))
        o2p = ctx.enter_context(tc.tile_pool(name="o2p", bufs=2))
        outp = ctx.enter_context(tc.tile_pool(name="outp", bufs=2))
        # psA: 2-deep ring of short-lived psum tiles; psB: attn.V accumulators
        psA = ctx.enter_context(tc.tile_pool(name="psA", bufs=2, space="PSUM"))
        psB = ctx.enter_context(tc.tile_pool(name="psB", bufs=2, space="PSUM"))

        # ---- input DMAs: x tiles spread over 3 queues, weights after ----
        x_sb = xp.tile([128, CT, HW], F32)
        x_r = x_d[:].rearrange("(t p) s -> t p s", p=128)
        for t, eng in zip(range(CT), (nc.sync, nc.scalar, nc.gpsimd, nc.sync)):
            eng.dma_start(out=x_sb[:, t, :], in_=x_r[t])
        gam_sb = const.tile([128, CT], F32, tag="gam")
        nc.scalar.dma_start(out=gam_sb[:], in_=gam_d[:].rearrange("(t p) -> p t", p=128))
        bet_sb = const.tile([128, CT], F32, tag="bet")
        nc.scalar.dma_start(out=bet_sb[:], in_=bet_d[:].rearrange("(t p) -> p t", p=128))
        qkb_sb = const.tile([128, 2 * CT], F32, tag="qkb")
        nc.scalar.dma_start(out=qkb_sb[:], in_=qkb_d[:].rearrange("(t p) -> p t", p=128))
        G_sb = const.tile([128, GPT], F32, tag="G")
        nc.scalar.dma_start(out=G_sb[:], in_=G_d[:])
        GT_sb = const.tile([8, 128], F32, tag="GT")
        nc.scalar.dma_start(out=GT_sb[:], in_=GT_d[:])
        B2_sb = const.tile([128, CT, DH], F32, tag="B2")
        nc.scalar.dma_start(out=B2_sb[:], in_=B2_d[:].rearrange("(t p) d -> p t d", p=128))
        wq_sb = wqp.tile([128, CT, 3 * C], F8)
        nc.gpsimd.dma_start(out=wq_sb[:], in_=wq_d[:].rearrange("(t p) o -> p t o", p=128))
        wp_sb = wpp.tile([128, CT, C], F8)
        nc.scalar.dma_start(out=wp_sb[:], in_=wp_d[:].rearrange("(t p) o -> p t o", p=128))

        # ---- persistent tiles + memsets (run during input DMA) ----
        # k per head-half with the other 64 partitions zero, so the scores
        # contraction is a full K=128 with a full-size weight tile:
        kA_sb = kpA.tile([128, CT, ST, 128], F8)
        nc.vector.memset(kA_sb[:].rearrange("p a b c -> p (a b c)"), 0.0)
        kB_sb = kpB.tile([128, CT, ST, 128], F8)
        nc.vector.memset(kB_sb[:].rearrange("p a b c -> p (a b c)"), 0.0)
        q_sb = qp.tile([128, CT, HW], F8)
        # v padded to 128 cols: [0:64]=v, 64=ones (row sums), 65:128=zero
        v_sb = vpl.tile([128, ST, NH, 128], F8)
        nc.vector.memset(v_sb[:].rearrange("p a b c -> p (a b c)"), 0.0)
        nc.vector.memset(v_sb[:, :, :, 64], 1.0)
        expb = const.tile([128, 1], F32, tag="expb")
        nc.vector.memset(expb[:], -float(EXPB))
        eps_sb = const.tile([8, 1], F32, tag="eps")
        nc.vector.memset(eps_sb[:], float(EPS))
        h2_sb = h2p.tile([128, CT, HW], F8)

        # preload ACT sqrt table while DMAs run
        dummy = small.tile([1, 1], F32, tag="dummy")
        nc.vector.memset(dummy[:], 1.0)
        dummy2 = small.tile([1, 1], F32, tag="dummy2")
        nc.scalar.activation(dummy2[:], dummy[:],
                             mybir.ActivationFunctionType.Sqrt, bias=0.0, scale=1.0)

        # ---- groupnorm ----
        mv = small.tile([128, CT, 3], F32, tag="mv")
        for t in range(CT):
            st = small.tile([128, 2, 6], F32, tag="bnst")
            x3 = x_sb[:, t, :].rearrange("p (a f) -> p a f", a=2)
            nc.vector.bn_stats(st[:, 0, :], x3[:, 0, :])
            nc.vector.bn_stats(st[:, 1, :], x3[:, 1, :])
            nc.vector.bn_aggr(mv[:, t, 0:2], st[:])
            nc.vector.tensor_mul(mv[:, t, 2:3], mv[:, t, 0:1], mv[:, t, 0:1])
        # PE warm-up: gated on a vector memset emitted after the bn chain, so
        # the dummy matmuls run while the groupnorm scalar chain resolves and
        # the PE enters the conv section at full clock
        wu_w = const.tile([128, 128], BF16, tag="wu_w")
        nc.vector.memset(wu_w[:], 0.0)
        wu_r = const.tile([128, 512], BF16, tag="wu_r")
        nc.vector.memset(wu_r[:], 0.0)
        for i in range(8):
            wps = psA.tile([128, 512], F32, tag="sc", name=f"wu{i}")
            nc.tensor.matmul(wps[:], lhsT=wu_w[:], rhs=wu_r[:],
                             start=True, stop=True)
        ps_g = psA.tile([8, CT * 3], F32, tag="sc", name="gn_g")
        nc.tensor.matmul(ps_g[:], lhsT=G_sb[:], rhs=mv[:].rearrange("p a b -> p (a b)"),
                         start=True, stop=True)
        gst = small.tile([8, CT, 3], F32, tag="gst")
        nc.vector.tensor_copy(gst[:].rearrange("p a b -> p (a b)"), ps_g[:])
        sq = small.tile([8, CT], F32, tag="sq")
        nc.vector.tensor_mul(sq[:], gst[:, :, 0], gst[:, :, 0])
        var4 = small.tile([8, CT], F32, tag="var4")
        nc.vector.tensor_add(var4[:], gst[:, :, 1], gst[:, :, 2])
        nc.vector.tensor_sub(var4[:], var4[:], sq[:])
        srt = small.tile([8, CT], F32, tag="srt")
        nc.scalar.activation(srt[:], var4[:], mybir.ActivationFunctionType.Sqrt,
                             bias=eps_sb[:], scale=1.0)
        # preload ACT exp table right after the sqrt (ScalarE idle otherwise)
        dummy3 = small.tile([1, 1], F32, tag="dummy3")
        nc.scalar.activation(dummy3[:], dummy[:],
                             mybir.ActivationFunctionType.Exp, scale=1.0)
        rstd = small.tile([8, CT], F32, tag="rstd")
        nc.vector.reciprocal(rstd[:], srt[:])
        gv2 = small.tile([8, CT, 2], F32, tag="gv2")
        nc.vector.tensor_copy(gv2[:, :, 0], rstd[:])
        nc.vector.tensor_copy(gv2[:, :, 1], gst[:, :, 0])
        ps_b = psA.tile([128, CT * 2], F32, tag="sc", name="gn_b")
        nc.tensor.matmul(ps_b[:], lhsT=GT_sb[:], rhs=gv2[:].rearrange("p a b -> p (a b)"),
                         start=True, stop=True)
        bc = small.tile([128, CT, 2], F32, tag="bc")
        nc.vector.tensor_copy(bc[:].rearrange("p a b -> p (a b)"), ps_b[:])
        # batched scale/shift: sc0 = rstd*gamma, sc1 = beta - mean*sc0
        scf = small.tile([128, CT, 2], F32, tag="scf")
        nc.vector.tensor_mul(scf[:, :, 0], bc[:, :, 0], gam_sb[:])
        nc.vector.tensor_mul(scf[:, :, 1], bc[:, :, 1], scf[:, :, 0])
        nc.vector.tensor_sub(scf[:, :, 1], bet_sb[:], scf[:, :, 1])
        h_sb = hp.tile([128, CT, HW], F8)
        for t in range(CT):
            eng = nc.vector if t % 2 == 0 else nc.gpsimd
            eng.tensor_scalar(
                out=h_sb[:, t, :], in0=x_sb[:, t, :],
                scalar1=scf[:, t, 0:1], scalar2=scf[:, t, 1:2],
                op0=mybir.AluOpType.mult, op1=mybir.AluOpType.add)

        # ---- qkv conv (DoubleRow fp8) as single-pass generators ----
        def emit_q_conv(m):
            ps = psA.tile([128, HW], F32, tag="sc", name=f"qps{m}")
            for t in range(2):
                for n in range(2):
                    yield nc.tensor.matmul(
                        ps[:, n * 512:(n + 1) * 512],
                        lhsT=wq_sb[:, 2 * t:2 * t + 2, m * 128:(m + 1) * 128],
                        rhs=h_sb[:, 2 * t:2 * t + 2, n * 512:(n + 1) * 512],
                        start=(t == 0), stop=(t == 1), perf_mode=DR)
            nc.vector.tensor_scalar_add(q_sb[:, m, :], ps[:], qkb_sb[:, m:m + 1])

        def emit_k_conv(m):
            ps = psA.tile([128, HW], F32, tag="sc", name=f"kps{m}")
            for t in range(2):
                for n in range(2):
                    yield nc.tensor.matmul(
                        ps[:, n * 512:(n + 1) * 512],
                        lhsT=wq_sb[:, 2 * t:2 * t + 2, C + m * 128:C + (m + 1) * 128],
                        rhs=h_sb[:, 2 * t:2 * t + 2, n * 512:(n + 1) * 512],
                        start=(t == 0), stop=(t == 1), perf_mode=DR)
            nc.vector.tensor_scalar_add(
                kA_sb[0:64, m, :, :],
                ps[0:64, :].rearrange("p (a b) -> p a b", a=ST),
                qkb_sb[0:64, CT + m:CT + m + 1])
            nc.vector.tensor_scalar_add(
                kB_sb[64:128, m, :, :],
                ps[64:128, :].rearrange("p (a b) -> p a b", a=ST),
                qkb_sb[64:128, CT + m:CT + m + 1])

        def emit_v_conv(m):
            psv = psA.tile([128, 512], F32, tag="sc", name=f"vps{m}")
            for t in range(2):
                yield nc.tensor.matmul(
                    psv[:], lhsT=h_sb[:, 2 * t:2 * t + 2, m * 128:(m + 1) * 128],
                    rhs=wq_sb[:, 2 * t:2 * t + 2, 2 * C:3 * C],
                    start=(t == 0), stop=(t == 1), perf_mode=DR)
            nc.vector.tensor_copy(
                v_sb[:, m, :, 0:64],
                psv[:].rearrange("p (h d) -> p h d", d=64))

        # q0/k0 fully before pair 0; the rest threads into the pair loop
        for _ in emit_q_conv(0):
            pass
        for _ in emit_k_conv(0):
            pass

        # fold B2 into x for the residual (gpsimd, idle mid-kernel)
        for t in range(CT):
            b2a = B2_sb[:, t, :]
            b2bc = bass.AP(tensor=b2a.tensor, offset=b2a.offset,
                           ap=[[b2a.ap[0][0], 128], [0, HW // DH], [1, DH]])
            nc.gpsimd.tensor_tensor(out=x_sb[:, t, :], in0=x_sb[:, t, :],
                                    in1=b2bc, op=mybir.AluOpType.add)

        # ---- attention pair loop with a single-pass FIFO work queue ----
        active_gens = []

        def pop_work(k):
            done = 0
            while done < k and active_gens:
                g = active_gens[0]
                try:
                    next(g)
                    done += 1
                except StopIteration:
                    active_gens.pop(0)

        for m in (1, 2, 3):
            active_gens.append(emit_q_conv(m))
            active_gens.append(emit_k_conv(m))
        for m in range(ST):
            active_gens.append(emit_v_conv(m))

        def gen_attnv_chunk(pt, po, h, half, t):
            for n in range(2):
                yield nc.tensor.matmul(
                    po[:, n * 512:(n + 1) * 512],
                    lhsT=v_sb[:, 2 * t:2 * t + 2, h, :],
                    rhs=pt[:, 2 * t:2 * t + 2, half, n * 512:(n + 1) * 512],
                    start=(t == 0), stop=(t == 3), perf_mode=DR)

        def gen_norm(p, h, half, po):
            # normalize po (attn out^T with row sums at partition 64),
            # transpose, scatter to h2 dram, read back channel-major
            o2t = o2tp.tile([80, HW], BF16, tag="o2t")
            nc.vector.tensor_copy(o2t[0:65, :], po[0:65, :])
            o2tr = o2trp.tile([128, ST, 80], BF16, tag="o2tr")
            nc.sync.dma_start_transpose(o2tr[:], o2t[:])
            linv = small.tile([128, ST], F32, tag="linv")
            nc.vector.reciprocal(linv[:], o2tr[:, :, 64])
            o2 = o2p.tile([128, 512], F8, tag="o2")
            lap = linv[:]
            lbc = bass.AP(tensor=lap.tensor, offset=lap.offset,
                          ap=[[lap.ap[0][0], 128], [1, ST], [0, 64]])
            nc.vector.tensor_mul(
                o2[:].rearrange("p (q d) -> p q d", d=64),
                o2tr[:, :, 0:64], lbc)
            wr = nc.gpsimd.dma_start(
                out=h2_d[:].rearrange("c s -> (c s)")
                [h * 65536:(h + 1) * 65536]
                .rearrange("(q p d) -> p q d", p=128, d=64),
                in_=o2[:].rearrange("p (q d) -> p q d", d=64))
            rd = nc.gpsimd.dma_start(
                out=h2_sb[64 * half:64 * half + 64, p, :],
                in_=h2_d[h * 64:(h + 1) * 64, :])
            bass_rust.add_dep_helper(rd.ins, wr.ins, reason="h2 RAW")
            yield rd

        for p in range(4):
            hA, hB = 2 * p, 2 * p + 1
            pt = ptp.tile([128, ST, 2, HW], F8, tag="pt", name=f"pt{p}")
            po_A = psB.tile([128, HW], F32, tag="att", name=f"poA{p}")
            po_B = psB.tile([128, HW], F32, tag="att", name=f"poB{p}")
            for jt in range(ST):
                for half, h in ((0, hA), (1, hB)):
                    k_sb = kA_sb if half == 0 else kB_sb
                    ps = psA.tile([128, HW], F32, tag="sc", name=f"sc{h}_{jt}")
                    for n in range(2):
                        nc.tensor.matmul(
                            ps[:, n * 512:(n + 1) * 512],
                            lhsT=k_sb[:, p, jt, :],
                            rhs=q_sb[:, p, n * 512:(n + 1) * 512],
                            start=True, stop=True)
                    nc.scalar.activation(pt[:, jt, half, :], ps[:],
                                         mybir.ActivationFunctionType.Exp,
                                         bias=expb[:], scale=float(DH ** -0.5))
                    pop_work(2)
                if jt % 2 == 1:
                    t = (jt - 1) // 2
                    active_gens.append(gen_attnv_chunk(pt, po_A, hA, 0, t))
                    active_gens.append(gen_attnv_chunk(pt, po_B, hB, 1, t))
                    if t == 3:
                        active_gens.append(gen_norm(p, hA, 0, po_A))
                        active_gens.append(gen_norm(p, hB, 1, po_B))

        # drain remaining work (tail of pair 3)
        pop_work(1000)

        # ---- proj (DoubleRow fp8) + residual + out ----
        for o in range(CT):
            pp = psA.tile([128, HW], F32, tag="sc", name=f"pp{o}")
            for t in range(2):
                for n in range(2):
                    nc.tensor.matmul(
                        pp[:, n * 512:(n + 1) * 512],
                        lhsT=wp_sb[:, 2 * t:2 * t + 2, o * 128:(o + 1) * 128],
                        rhs=h2_sb[:, 2 * t:2 * t + 2, n * 512:(n + 1) * 512],
                        start=(t == 0), stop=(t == 1), perf_mode=DR)
            ot = outp.tile([128, HW], F32, tag="ot")
            nc.vector.tensor_add(ot[:], pp[:], x_sb[:, o, :])
            nc.sync.dma_start(out=out_d[o * 128:(o + 1) * 128, 0:512],
                              in_=ot[:, 0:512])
            nc.scalar.dma_start(out=out_d[o * 128:(o + 1) * 128, 512:1024],
                                in_=ot[:, 512:1024])

    nc.compile()
    return nc


def _host_prep(x, norm_gamma, norm_beta, qkv_w, qkv_b, proj_w, proj_b):
    x = np.asarray(x, dtype=np.float32).reshape(B, C, HW)
    qkv_w = np.asarray(qkv_w, dtype=np.float32)
    qkv_b = np.asarray(qkv_b, dtype=np.float32)
    proj_w = np.asarray(proj_w, dtype=np.float32)
    proj_b = np.asarray(proj_b, dtype=np.float32)

    wqkvT = np.ascontiguousarray(qkv_w.T).astype(NPF8)
    wprojT = np.ascontiguousarray(proj_w.T).astype(NPF8)
    qkb = np.ascontiguousarray(qkv_b[:2 * C])
    vb = qkv_b[2 * C:].astype(np.float64)          # [512]
    # B2[o, m] = proj_b[o] + sum_h (sum_{c' in head h} proj_w[o, 64h+c']) * vb[64h+m]
    psum_h = proj_w.astype(np.float64).reshape(C, NH, DH).sum(axis=2)   # [o, h]
    vb_hm = vb.reshape(NH, DH)                                          # [h, m]
    B2 = (proj_b.astype(np.float64)[:, None] + psum_h @ vb_hm).astype(np.float32)

    G = np.zeros((128, GPT), np.float32)
    for p in range(128):
        G[p, p // CPG] = 1.0 / CPG
    GT = np.zeros((8, 128), np.float32)
    for p in range(128):
        GT[p // CPG, p] = 1.0

    gamma = np.ascontiguousarray(norm_gamma, dtype=np.float32)
    beta = np.ascontiguousarray(norm_beta, dtype=np.float32)

    in_maps = []
    for b in range(B):
        in_maps.append({
            "x": np.ascontiguousarray(x[b]),
            "wqkvT": wqkvT, "wprojT": wprojT,
            "qkb": qkb, "gamma": gamma, "beta": beta,
            "G": G, "GT": GT, "B2": B2,
        })
    return in_maps


def _run(inputs: dict, trace: bool = False, tmpdir=None):
    if "nc" not in _CACHE:
        _CACHE["nc"] = _build()
    nc = _CACHE["nc"]
    in_maps = _host_prep(**inputs)
    res = run_bass_kernel_spmd(nc, in_maps, core_ids=list(range(8)), trace=trace,
                               tmpdir=tmpdir)
    out = np.stack([r["out"] for r in res.results]).reshape(B, C, 32, 32)
    return out.astype(np.float32), res


def kernel(**inputs):
    out, _ = _run(inputs, trace=False)
    return out


# revision 9
# speedup vs baseline: 1.0662x; 1.0662x over previous
"""Trainium2 Bass kernel for nn_AttentionBlock (B=8, C=512, H=W=32, NH=8, DH=64).

Sharding: pure data-parallel - one batch element per NeuronCore (8 cores).

Per-core pipeline (channels-on-partitions, HW=1024 spatial), fp8-heavy:
  groupnorm (f32 stats, h cast to fp8e4)
  -> qkv 1x1conv as fp8 DoubleRow matmuls (K=256/pass, 2 cols/cycle)
  -> attention:
       scores S^T[j,i] = k_j . q_i as plain fp8 matmuls with the 64-dim
       head contraction zero-padded to K=128 (kA rows 64-127 = 0, kB
       rows 0-63 = 0): every weight load is a full 128x128 tile at a
       fixed PE position, so it pipelines behind the previous stream;
       exp on ScalarE (scale=1/8, bias=-2; the bias cancels in softmax
       and keeps exp outputs inside fp8e4 range) writing fp8 directly;
       attn.V as fp8 DoubleRow over spatial-tile pairs, V zero-padded
       to 128 columns with a ones-column at 64 giving row sums l;
       transpose via DMA xbar, normalize on VectorE -> h2 (fp8) via
       DRAM round-trip
  -> proj 1x1conv fp8 DoubleRow -> residual add (x + B2 bias fold).

ScalarE exp (64 x [128,1024] tiles, ~71us) is the pipeline floor. All
other tensor work (qkv conv leftovers, attn.V passes, normalize steps)
is threaded one matmul at a time into the slack between score matmuls
via a FIFO work queue (2 pops per exp), so the exp stream stays dense.
PSUM: psA = 2-deep ring of short-lived tiles (warmup/gn/conv/scores/
proj), psB = the two long-lived attn.V accumulators of the active pair.

Host-side: weights transposed + cast to fp8e4; v-bias and proj-bias
folded into B2[o, s%64] added into x (gpsimd) for the residual.
"""

import numpy as np
import ml_dtypes

import concourse.bass as bass
import concourse.mybir as mybir
import concourse.tile as tile
from concourse import bacc
from concourse.bass_utils import run_bass_kernel_spmd

F32 = mybir.dt.float32
BF16 = mybir.dt.bfloat16
F8 = mybir.dt.float8e4
DR = mybir.MatmulPerfMode.DoubleRow
NPF8 = ml_dtypes.float8_e4m3

B, C, HW = 8, 512, 1024
NH, DH = 8, 64
GROUPS, EPS = 32, 1e-5
CT = C // 128          # 4 channel tiles
ST = HW // 128         # 8 spatial tiles
GPT = 8                # groups per 128-channel tile
CPG = 16               # channels per group
EXPB = 2.0             # exp bias (cancels in softmax; keeps fp8 in range)

_CACHE: dict = {}


def _build():
    nc = bacc.Bacc("TRN2", target_bir_lowering=False, debug=False, num_devices=8)

    x_d = nc.declare_dram_parameter("x", [C, HW], F32, isOutput=False)
    wq_d = nc.declare_dram_parameter("wqkvT", [C, 3 * C], F8, isOutput=False)
    wp_d = nc.declare_dram_parameter("wprojT", [128, 2048], F8, isOutput=False)
    qkb_d = nc.declare_dram_parameter("qkb", [2 * C], F32, isOutput=False)
    gam_d = nc.declare_dram_parameter("gamma", [C], F32, isOutput=False)
    bet_d = nc.declare_dram_parameter("beta", [C], F32, isOutput=False)
    G_d = nc.declare_dram_parameter("G", [128, GPT], F32, isOutput=False)
    GT_d = nc.declare_dram_parameter("GT", [8, 128], F32, isOutput=False)
    B2_d = nc.declare_dram_parameter("B2", [C, DH], F32, isOutput=False)
    out_d = nc.declare_dram_parameter("out", [C, HW], F32, isOutput=True)
    h2_d = nc.dram_tensor("h2d", [C, HW], F8)

    import bass_rust
    from contextlib import ExitStack

    with tile.TileContext(nc) as tc, ExitStack() as ctx:
        const = ctx.enter_context(tc.tile_pool(name="const", bufs=1))
        small = ctx.enter_context(tc.tile_pool(name="small", bufs=2))
        xp = ctx.enter_context(tc.tile_pool(name="xp", bufs=1))
        hp = ctx.enter_context(tc.tile_pool(name="hp", bufs=1))
        wqp = ctx.enter_context(tc.tile_pool(name="wqp", bufs=1))
        wpp = ctx.enter_context(tc.tile_pool(name="wpp", bufs=1))
        qp = ctx.enter_context(tc.tile_pool(name="qp", bufs=1))
        kpA = ctx.enter_context(tc.tile_pool(name="kpA", bufs=1))
        kpB = ctx.enter_context(tc.tile_pool(name="kpB", bufs=1))
        vpl = ctx.enter_context(tc.tile_pool(name="vpl", bufs=1))
        ptp = ctx.enter_context(tc.tile_pool(name="ptp", bufs=2))
        h2p = ctx.enter_context(tc.tile_pool(name="h2p", bufs=1))
        o2tp = ctx.enter_context(tc.tile_pool(name="o2tp", bufs=2))
        o2trp = ctx.enter_context(tc.tile_pool(name="o2trp", bufs=2))
        o2p = ctx.enter_context(tc.tile_pool(name="o2p", bufs=2))
        outp = ctx.enter_context(tc.tile_pool(name="outp", bufs=2))
        # psA: 2-deep ring of short-lived psum tiles; psB: attn.V accumulators
        psA = ctx.enter_context(tc.tile_pool(name="psA", bufs=2, space="PSUM"))
        psB = ctx.enter_context(tc.tile_pool(name="psB", bufs=2, space="PSUM"))

        # ---- input DMAs: x tiles spread over 3 queues, weights after ----
        x_sb = xp.tile([128, CT, HW], F32)
        x_r = x_d[:].rearrange("(t p) s -> t p s", p=128)
        for t, eng in zip(range(CT), (nc.sync, nc.scalar, nc.gpsimd, nc.sync)):
            eng.dma_start(out=x_sb[:, t, :], in_=x_r[t])
        gam_sb = const.tile([128, CT], F32, tag="gam")
        nc.scalar.dma_start(out=gam_sb[:], in_=gam_d[:].rearrange("(t p) -> p t", p=128))
        bet_sb = const.tile([128, CT], F32, tag="bet")
        nc.scalar.dma_start(out=bet_sb[:], in_=bet_d[:].rearrange("(t p) -> p t", p=128))
        qkb_sb = const.tile([128, 2 * CT], F32, tag="qkb")
        nc.scalar.dma_start(out=qkb_sb[:], in_=qkb_d[:].rearrange("(t p) -> p t", p=128))
        G_sb = const.tile([128, GPT], F32, tag="G")
        nc.scalar.dma_start(out=G_sb[:], in_=G_d[:])
        GT_sb = const.tile([8, 128], F32, tag="GT")
        nc.scalar.dma_start(out=GT_sb[:], in_=GT_d[:])
        B2_sb = const.tile([128, CT, DH], F32, tag="B2")
        nc.scalar.dma_start(out=B2_sb[:], in_=B2_d[:].rearrange("(t p) d -> p t d", p=128))
        wq_sb = wqp.tile([128, CT, 3 * C], F8)
        nc.gpsimd.dma_start(out=wq_sb[:], in_=wq_d[:].rearrange("(t p) o -> p t o", p=128))
        # wp pre-tiled on host: [p, kt_pair, o_tile, sub, 128]
        wp_sb = wpp.tile([128, 2, CT, 2, 128], F8)
        nc.scalar.dma_start(out=wp_sb[:].rearrange("p a b c d -> p (a b c d)"),
                            in_=wp_d[:])

        # ---- persistent tiles + memsets (run during input DMA) ----
        # k per head-half with the other 64 partitions zero, so the scores
        # contraction is a full K=128 with a full-size weight tile:
        kA_sb = kpA.tile([128, CT, ST, 128], F8)
        nc.gpsimd.memset(kA_sb[64:128, :, :, :].rearrange("p a b c -> p (a b) c"), 0.0)
        kB_sb = kpB.tile([128, CT, ST, 128], F8)
        nc.gpsimd.memset(kB_sb[0:64, :, :, :].rearrange("p a b c -> p (a b) c"), 0.0)
        q_sb = qp.tile([128, CT, HW], F8)
        # v padded to 128 cols: [0:64]=v, 64=ones (row sums), 65:128=zero
        v_sb = vpl.tile([128, ST, NH, 128], F8)
        nc.gpsimd.memset(v_sb[:, :, :, 64:128].rearrange("p a b c -> p (a b) c"), 0.0)
        nc.gpsimd.memset(v_sb[:, :, :, 64], 1.0)
        expb = const.tile([128, 1], F32, tag="expb")
        nc.vector.memset(expb[:], -float(EXPB))
        eps_sb = const.tile([8, 1], F32, tag="eps")
        nc.vector.memset(eps_sb[:], float(EPS))
        h2_sb = h2p.tile([128, CT, HW], F8)

        # preload ACT sqrt table while DMAs run
        dummy = small.tile([1, 1], F32, tag="dummy")
        nc.vector.memset(dummy[:], 1.0)
        dummy2 = small.tile([1, 1], F32, tag="dummy2")
        nc.scalar.activation(dummy2[:], dummy[:],
                             mybir.ActivationFunctionType.Sqrt, bias=0.0, scale=1.0)

        # ---- groupnorm ----
        mv = small.tile([128, CT, 3], F32, tag="mv")
        for t in range(CT):
            st = small.tile([128, 2, 6], F32, tag="bnst")
            x3 = x_sb[:, t, :].rearrange("p (a f) -> p a f", a=2)
            nc.vector.bn_stats(st[:, 0, :], x3[:, 0, :])
            nc.vector.bn_stats(st[:, 1, :], x3[:, 1, :])
            nc.vector.bn_aggr(mv[:, t, 0:2], st[:])
            nc.vector.tensor_mul(mv[:, t, 2:3], mv[:, t, 0:1], mv[:, t, 0:1])
        # PE warm-up: gated on a vector memset emitted after the bn chain, so
        # the dummy matmuls run while the groupnorm scalar chain resolves and
        # the PE enters the conv section at full clock
        wu_w = const.tile([128, 128], BF16, tag="wu_w")
        nc.gpsimd.memset(wu_w[:], 0.0)
        wu_r = const.tile([128, 512], BF16, tag="wu_r")
        nc.gpsimd.memset(wu_r[:], 0.0)
        for i in range(8):
            wps = psA.tile([128, 512], F32, tag="sc", name=f"wu{i}")
            nc.tensor.matmul(wps[:], lhsT=wu_w[:], rhs=wu_r[:],
                             start=True, stop=True)
        ps_g = psA.tile([8, CT * 3], F32, tag="sc", name="gn_g")
        nc.tensor.matmul(ps_g[:], lhsT=G_sb[:], rhs=mv[:].rearrange("p a b -> p (a b)"),
                         start=True, stop=True)
        gst = small.tile([8, CT, 3], F32, tag="gst")
        nc.vector.tensor_copy(gst[:].rearrange("p a b -> p (a b)"), ps_g[:])
        sq = small.tile([8, CT], F32, tag="sq")
        nc.vector.tensor_mul(sq[:], gst[:, :, 0], gst[:, :, 0])
        var4 = small.tile([8, CT], F32, tag="var4")
        nc.vector.tensor_add(var4[:], gst[:, :, 1], gst[:, :, 2])
        nc.vector.tensor_sub(var4[:], var4[:], sq[:])
        srt = small.tile([8, CT], F32, tag="srt")
        nc.scalar.activation(srt[:], var4[:], mybir.ActivationFunctionType.Sqrt,
                             bias=eps_sb[:], scale=1.0)
        # preload ACT exp table right after the sqrt (ScalarE idle otherwise)
        dummy3 = small.tile([1, 1], F32, tag="dummy3")
        nc.scalar.activation(dummy3[:], dummy[:],
                             mybir.ActivationFunctionType.Exp, scale=1.0)
        rstd = small.tile([8, CT], F32, tag="rstd")
        nc.vector.reciprocal(rstd[:], srt[:])
        gv2 = small.tile([8, CT, 2], F32, tag="gv2")
        nc.vector.tensor_copy(gv2[:, :, 0], rstd[:])
        nc.vector.tensor_copy(gv2[:, :, 1], gst[:, :, 0])
        ps_b = psA.tile([128, CT * 2], F32, tag="sc", name="gn_b")
        nc.tensor.matmul(ps_b[:], lhsT=GT_sb[:], rhs=gv2[:].rearrange("p a b -> p (a b)"),
                         start=True, stop=True)
        bc = small.tile([128, CT, 2], F32, tag="bc")
        nc.vector.tensor_copy(bc[:].rearrange("p a b -> p (a b)"), ps_b[:])
        # batched scale/shift: sc0 = rstd*gamma, sc1 = beta - mean*sc0
        scf = small.tile([128, CT, 2], F32, tag="scf")
        nc.vector.tensor_mul(scf[:, :, 0], bc[:, :, 0], gam_sb[:])
        nc.vector.tensor_mul(scf[:, :, 1], bc[:, :, 1], scf[:, :, 0])
        nc.vector.tensor_sub(scf[:, :, 1], bet_sb[:], scf[:, :, 1])
        h_sb = hp.tile([128, CT, HW], F8)
        for t in range(CT):
            eng = nc.vector if t % 2 == 0 else nc.gpsimd
            eng.tensor_scalar(
                out=h_sb[:, t, :], in0=x_sb[:, t, :],
                scalar1=scf[:, t, 0:1], scalar2=scf[:, t, 1:2],
                op0=mybir.AluOpType.mult, op1=mybir.AluOpType.add)

        # ---- qkv conv (DoubleRow fp8) as single-pass generators ----
        def emit_q_conv(m):
            ps = psA.tile([128, HW], F32, tag="sc", name=f"qps{m}")
            for t in range(2):
                for n in range(2):
                    yield nc.tensor.matmul(
                        ps[:, n * 512:(n + 1) * 512],
                        lhsT=wq_sb[:, 2 * t:2 * t + 2, m * 128:(m + 1) * 128],
                        rhs=h_sb[:, 2 * t:2 * t + 2, n * 512:(n + 1) * 512],
                        start=(t == 0), stop=(t == 1), perf_mode=DR)
            nc.vector.tensor_scalar_add(q_sb[:, m, :], ps[:], qkb_sb[:, m:m + 1])

        def emit_k_conv(m):
            ps = psA.tile([128, HW], F32, tag="sc", name=f"kps{m}")
            for t in range(2):
                for n in range(2):
                    yield nc.tensor.matmul(
                        ps[:, n * 512:(n + 1) * 512],
                        lhsT=wq_sb[:, 2 * t:2 * t + 2, C + m * 128:C + (m + 1) * 128],
                        rhs=h_sb[:, 2 * t:2 * t + 2, n * 512:(n + 1) * 512],
                        start=(t == 0), stop=(t == 1), perf_mode=DR)
            nc.vector.tensor_scalar_add(
                kA_sb[0:64, m, :, :],
                ps[0:64, :].rearrange("p (a b) -> p a b", a=ST),
                qkb_sb[0:64, CT + m:CT + m + 1])
            nc.vector.tensor_scalar_add(
                kB_sb[64:128, m, :, :],
                ps[64:128, :].rearrange("p (a b) -> p a b", a=ST),
                qkb_sb[64:128, CT + m:CT + m + 1])

        def emit_v_conv(m):
            psv = psA.tile([128, 512], F32, tag="sc", name=f"vps{m}")
            for t in range(2):
                yield nc.tensor.matmul(
                    psv[:], lhsT=h_sb[:, 2 * t:2 * t + 2, m * 128:(m + 1) * 128],
                    rhs=wq_sb[:, 2 * t:2 * t + 2, 2 * C:3 * C],
                    start=(t == 0), stop=(t == 1), perf_mode=DR)
            nc.vector.tensor_copy(
                v_sb[:, m, :, 0:64],
                psv[:].rearrange("p (h d) -> p h d", d=64))

        # q0/k0 fully before pair 0; the rest threads into the pair loop
        for _ in emit_q_conv(0):
            pass
        for _ in emit_k_conv(0):
            pass

        # fold B2 into x for the residual (gpsimd, idle mid-kernel)
        for t in range(CT):
            b2a = B2_sb[:, t, :]
            b2bc = bass.AP(tensor=b2a.tensor, offset=b2a.offset,
                           ap=[[b2a.ap[0][0], 128], [0, HW // DH], [1, DH]])
            nc.gpsimd.tensor_tensor(out=x_sb[:, t, :], in0=x_sb[:, t, :],
                                    in1=b2bc, op=mybir.AluOpType.add)

        # ---- attention pair loop with a single-pass FIFO work queue ----
        active_gens = []

        def pop_work(k):
            done = 0
            while done < k and active_gens:
                g = active_gens[0]
                try:
                    next(g)
                    done += 1
                except StopIteration:
                    active_gens.pop(0)

        for m in (1, 2, 3):
            active_gens.append(emit_q_conv(m))
            active_gens.append(emit_k_conv(m))
        for m in range(ST):
            active_gens.append(emit_v_conv(m))

        def gen_attnv_chunk(pt, po, h, half, t):
            for n in range(2):
                yield nc.tensor.matmul(
                    po[:, n * 512:(n + 1) * 512],
                    lhsT=v_sb[:, 2 * t:2 * t + 2, h, :],
                    rhs=pt[:, 2 * t:2 * t + 2, half, n * 512:(n + 1) * 512],
                    start=(t == 0), stop=(t == 3), perf_mode=DR)

        def gen_norm(p, h, half, po):
            # normalize po (attn out^T with row sums at partition 64),
            # transpose, scatter to h2 dram, read back channel-major
            o2t = o2tp.tile([80, HW], BF16, tag="o2t")
            nc.vector.tensor_copy(o2t[0:65, :], po[0:65, :])
            o2tr = o2trp.tile([128, ST, 80], BF16, tag="o2tr")
            nc.sync.dma_start_transpose(o2tr[:], o2t[:])
            linv = small.tile([128, ST], F32, tag="linv")
            nc.vector.reciprocal(linv[:], o2tr[:, :, 64])
            o2 = o2p.tile([128, 512], F8, tag="o2")
            lap = linv[:]
            lbc = bass.AP(tensor=lap.tensor, offset=lap.offset,
                          ap=[[lap.ap[0][0], 128], [1, ST], [0, 64]])
            nc.vector.tensor_mul(
                o2[:].rearrange("p (q d) -> p q d", d=64),
                o2tr[:, :, 0:64], lbc)
            wr = nc.gpsimd.dma_start(
                out=h2_d[:].rearrange("c s -> (c s)")
                [h * 65536:(h + 1) * 65536]
                .rearrange("(q p d) -> p q d", p=128, d=64),
                in_=o2[:].rearrange("p (q d) -> p q d", d=64))
            rd = nc.gpsimd.dma_start(
                out=h2_sb[64 * half:64 * half + 64, p, :],
                in_=h2_d[h * 64:(h + 1) * 64, :])
            bass_rust.add_dep_helper(rd.ins, wr.ins, reason="h2 RAW")
            yield rd

        for p in range(4):
            hA, hB = 2 * p, 2 * p + 1
            pt = ptp.tile([128, ST, 2, HW], F8, tag="pt", name=f"pt{p}")
            po_A = psB.tile([128, HW], F32, tag="att", name=f"poA{p}")
            po_B = psB.tile([128, HW], F32, tag="att", name=f"poB{p}")
            for jt in range(ST):
                for half, h in ((0, hA), (1, hB)):
                    k_sb = kA_sb if half == 0 else kB_sb
                    ps = psA.tile([128, HW], F32, tag="sc", name=f"sc{h}_{jt}")
                    for n in range(2):
                        nc.tensor.matmul(
                            ps[:, n * 512:(n + 1) * 512],
                            lhsT=k_sb[:, p, jt, :],
                            rhs=q_sb[:, p, n * 512:(n + 1) * 512],
                            start=True, stop=True)
                    nc.scalar.activation(pt[:, jt, half, :], ps[:],
                                         mybir.ActivationFunctionType.Exp,
                                         bias=expb[:], scale=float(DH ** -0.5))
                    pop_work(2)
                if jt % 2 == 1:
                    t = (jt - 1) // 2
                    active_gens.append(gen_attnv_chunk(pt, po_A, hA, 0, t))
                    active_gens.append(gen_attnv_chunk(pt, po_B, hB, 1, t))
                    if t == 3:
                        active_gens.append(gen_norm(p, hA, 0, po_A))
                        active_gens.append(gen_norm(p, hB, 1, po_B))

        # drain remaining work (tail of pair 3)
        pop_work(1000)

        # ---- proj (DoubleRow fp8) + residual + out ----
        # kp0 passes (h2 tiles 0,1 = heads 0-3) run during the pair-3 norm
        # window; kp1 waits on the last h2 read-back
        pps = []
        for o in range(CT):
            pool = psA if o < 2 else psB
            pp = pool.tile([128, HW], F32, tag="sc" if o < 2 else "att",
                           name=f"pp{o}")
            pps.append(pp)
            for n in range(2):
                nc.tensor.matmul(
                    pp[:, n * 512:(n + 1) * 512],
                    lhsT=wp_sb[:, 0, o, :, :],
                    rhs=h2_sb[:, 0:2, n * 512:(n + 1) * 512],
                    start=True, stop=False, perf_mode=DR)
        for o in range(CT):
            pp = pps[o]
            for n in range(2):
                nc.tensor.matmul(
                    pp[:, n * 512:(n + 1) * 512],
                    lhsT=wp_sb[:, 1, o, :, :],
                    rhs=h2_sb[:, 2:4, n * 512:(n + 1) * 512],
                    start=False, stop=True, perf_mode=DR)
            ot = outp.tile([128, HW], F32, tag="ot")
            nc.vector.tensor_add(ot[:], pp[:], x_sb[:, o, :])
            nc.sync.dma_start(out=out_d[o * 128:(o + 1) * 128, 0:512],
                              in_=ot[:, 0:512])
            nc.scalar.dma_start(out=out_d[o * 128:(o + 1) * 128, 512:1024],
                                in_=ot[:, 512:1024])

    nc.compile()
    return nc


def _host_prep(x, norm_gamma, norm_beta, qkv_w, qkv_b, proj_w, proj_b):
    x = np.asarray(x, dtype=np.float32).reshape(B, C, HW)
    qkv_w = np.asarray(qkv_w, dtype=np.float32)
    qkv_b = np.asarray(qkv_b, dtype=np.float32)
    proj_w = np.asarray(proj_w, dtype=np.float32)
    proj_b = np.asarray(proj_b, dtype=np.float32)

    wqkvT = np.ascontiguousarray(qkv_w.T).astype(NPF8)
    # wprojT host-tiled to [p, kt_pair, o_tile, sub, j] flattened per p row
    wpT = proj_w.T.reshape(2, 2, 128, 4, 128)      # [tp, sub, p, o_tile, j]
    wprojT = np.ascontiguousarray(
        wpT.transpose(2, 0, 3, 1, 4).reshape(128, 2 * 4 * 2 * 128)).astype(NPF8)
    qkb = np.ascontiguousarray(qkv_b[:2 * C])
    vb = qkv_b[2 * C:].astype(np.float64)          # [512]
    # B2[o, m] = proj_b[o] + sum_h (sum_{c' in head h} proj_w[o, 64h+c']) * vb[64h+m]
    psum_h = proj_w.astype(np.float64).reshape(C, NH, DH).sum(axis=2)   # [o, h]
    vb_hm = vb.reshape(NH, DH)                                          # [h, m]
    B2 = (proj_b.astype(np.float64)[:, None] + psum_h @ vb_hm).astype(np.float32)

    G = np.zeros((128, GPT), np.float32)
    for p in range(128):
        G[p, p // CPG] = 1.0 / CPG
    GT = np.zeros((8, 128), np.float32)
    for p in range(128):
        GT[p // CPG, p] = 1.0

    gamma = np.ascontiguousarray(norm_gamma, dtype=np.float32)
    beta = np.ascontiguousarray(norm_beta, dtype=np.float32)

    in_maps = []
    for b in range(B):
        in_maps.append({
            "x": np.ascontiguousarray(x[b]),
            "wqkvT": wqkvT, "wprojT": wprojT,
            "qkb": qkb, "gamma": gamma, "beta": beta,
            "G": G, "GT": GT, "B2": B2,
        })
    return in_maps


def _run(inputs: dict, trace: bool = False, tmpdir=None):
    if "nc" not in _CACHE:
        _CACHE["nc"] = _build()
    nc = _CACHE["nc"]
    in_maps = _host_prep(**inputs)
    res = run_bass_kernel_spmd(nc, in_maps, core_ids=list(range(8)), trace=trace,
                               tmpdir=tmpdir)
    out = np.stack([r["out"] for r in res.results]).reshape(B, C, 32, 32)
    return out.astype(np.float32), res


def kernel(**inputs):
    out, _ = _run(inputs, trace=False)
    return out
